# revision 16
# baseline (speedup 1.0000x reference)
"""BERT-embedding kernel for Trainium2 (8 NeuronCores, data-parallel).

Computes, for input_sequence [256,512,10], doy_sequence [256,512] (int32),
W [256,10], b [256]:

    obs = input_sequence @ W.T + b          # [256,512,256]
    pos = PE_TABLE[doy_sequence]            # [256,512,256]
    out = concat([obs, pos], axis=-1)       # [256,512,512] fp32

Strategy: shard the batch dim 8 ways (32 batches / 16384 tokens per core),
replicate W/b and the 367x256 sinusoidal PE table. The kernel is HBM-write
bound (33.5 MB of output per core) and the tensor engine is per-instruction
bound (~320 ns/matmul regardless of streamed rows, DVFS-throttled), so the
design folds the ENTIRE per-128-token computation into exactly TWO fp8
DoubleRow matmuls (2 K-tiles each = 4 K-tiles of 128 partitions):

  - K-tiles 0,1 both carry [xh; xl] fp8 at partitions 106..127 (one
    window DMA per chunk, no pad); their rhs rows are
    [wh; wl] and [wl; wh] in columns 0:256, so the two tiles sum to all
    four cross terms = (xh+xl)@(wh+wl) = obs to ~1.6e-2 absolute.
  - The 367-row table re-packs as 106/106/128/128 rows on K-tiles 0..3
    starting at partition 0 (>32-partition engine APs must start at
    partition 0; tiles 2,3 are fully compare-written so no zero pad is
    needed anywhere). The rhs rows carry the fp8 PE table in columns
    256:512 (quantization error <= 0.031 absolute = 3.9e-3 of output
    absmax; gate is 2e-2).
  - The one-hot is built by gpsimd partition_broadcast of doy (fp16)
    against per-partition row ids (DVE is_equal -> fp8; dead slots
    compare against 1000). Chunks 0..3 ship pre-broadcast from the host
    so nothing waits on the Q7 library IRAM load. The x hi/lo rows land
    in partitions 0..10 of the same SBUF tile via one small DRAM DMA per
    1024-token chunk (no engine time).
  - PSUM holds 2 half-tiles [128, 4*512] (4 banks each); the scalar
    engine drains each with ONE [128, 4, 512] copy (the vector engine
    does only compares, so next-chunk one-hots never queue behind
    copies), then a token-major 1 MB HWDGE DMA per 512 tokens
    (sync/scalar queues) writes rows as contiguous 2 KB descriptors.

Per-core DMA is ~35.5 MB ~= the output-write roofline; PE does 16
matmuls per 1024 tokens = 256 total.
"""

import math

import numpy as np

import concourse.bacc as bacc
import concourse.mybir as mybir
import concourse.tile as tile
from concourse.bass_utils import run_bass_kernel_spmd
from concourse.library_config import mlp

F32 = mybir.dt.float32
F16 = mybir.dt.float16
F8 = mybir.dt.float8e4

# Problem shapes (hardcoded per the harness contract).
B, S, NF = 256, 512, 10
E = 256
MAX_LEN = 366
N_CORES = 8
TOK = (B // N_CORES) * S          # tokens per core = 16384
CH = 1024                          # tokens per chunk
GRP = CH // 128                    # 128-token groups per chunk = 8
NCH = TOK // CH                    # 16
KF = NF + 1                        # obs rows (features + bias ones-row)
XP = 2 * KF                        # x rows ([xh; xl], no pad)
XB = 106                           # base partition of the x rows
NKT = 4                            # K-tiles (2 DoubleRow matmuls)
RPT = (106, 106, 128, 128)         # table rows per K-tile (base partition 0)
ROFF = (0, 106, 212, 340)          # first table row of each K-tile
NROW = 468                         # padded table rows (367 used)
NHOST = 2                          # chunks whose doy-broadcast ships from host

_COMPILED_NC = None
_LAST_RESULTS = None               # BassKernelResults of the most recent run


def _make_pe() -> np.ndarray:
    """Sinusoidal table, row 0 zeros (padding), rows 1..366 = positions 0..365."""
    pe = np.zeros((NROW, E), dtype=np.float32)
    position = np.arange(0, MAX_LEN, dtype=np.float32)[:, None]
    div_term = np.exp(
        np.arange(0, E, 2, dtype=np.float32) * -(math.log(10000.0) / E)
    )
    pe[1 : MAX_LEN + 1, 0::2] = np.sin(position * div_term)
    pe[1 : MAX_LEN + 1, 1::2] = np.cos(position * div_term)
    return pe


def _build():
    nc = bacc.Bacc("TRN2", target_bir_lowering=False, debug=False)
    xhl = nc.dram_tensor("xhl", [XP, 2, TOK], F8, kind="ExternalInput")
    ohp0 = nc.dram_tensor("ohp0", [128, NKT, CH], F8, kind="ExternalInput")
    peW = nc.dram_tensor("peW", [128, NKT, 2 * E], F8, kind="ExternalInput")
    doy = nc.dram_tensor("doy", [1, TOK], F16, kind="ExternalInput")
    doyb0 = nc.dram_tensor("doyb0", [128, NHOST * CH], F16, kind="ExternalInput")
    cmp = nc.dram_tensor("cmp", [128, NKT], F32, kind="ExternalInput")
    out = nc.dram_tensor("out", [TOK, 2 * E], F32, kind="ExternalOutput")

    # out viewed as [chunk, half, partition, group, 512]:
    # token ((c*2+h)*4+j)*128+p
    out5 = out.ap().rearrange("(c h j p) e -> c h p j e", p=128, j=GRP // 2, h=2)

    with tile.TileContext(nc) as tc:
        with (
            tc.tile_pool(name="const", bufs=1) as const_pool,
            tc.tile_pool(name="doyb", bufs=5) as doyb_pool,
            tc.tile_pool(name="oh", bufs=3) as oh_pool,
            tc.tile_pool(name="ot", bufs=6) as ot_pool,
            tc.tile_pool(name="ps", bufs=2, space="PSUM") as ps_pool,
        ):
            # Q7 ucode for partition_broadcast; IRAM DMA overlaps the loads.
            nc.gpsimd.load_library(mlp)

            cmp_sb = const_pool.tile([128, NKT], F32, tag="cmp_sb")
            nc.scalar.dma_start(out=cmp_sb[:], in_=cmp[:, :])
            peW_sb = const_pool.tile([128, NKT, 2 * E], F8, tag="peW_sb")
            nc.scalar.dma_start(out=peW_sb[:], in_=peW[:, :, :])
            doyb0_sb = const_pool.tile([128, NHOST * CH], F16, tag="doyb0_sb")
            nc.scalar.dma_start(out=doyb0_sb[:], in_=doyb0[:, :])
            doy_sb = const_pool.tile([1, TOK], F16, tag="doy_sb")
            nc.sync.dma_start(out=doy_sb[:], in_=doy[:, :])

            for c in range(NCH):
                oh = oh_pool.tile([128, NKT, CH], F8, tag="oh")
                if c == 0:
                    # Chunk 0's whole K-tile (onehot + x rows) ships
                    # host-built: one DMA, no compare latency at startup.
                    nc.sync.dma_start(out=oh[:], in_=ohp0[:, :, :])
                else:
                    # doyb[p, t] = doy[t] for the chunk's tokens, fp16.
                    if c - 1 < NHOST:
                        doyb = doyb0_sb[:, (c - 1) * CH : c * CH]
                    else:
                        dt = doyb_pool.tile([128, CH], F16, tag="doyb")
                        nc.gpsimd.partition_broadcast(
                            dt[:], doy_sb[0:1, c * CH : (c + 1) * CH]
                        )
                        doyb = dt[:]
                    # [xh; xl] fp8 rows into partitions 106..127 of
                    # K-tiles 0,1 (pure DMA).
                    nc.sync.dma_start(
                        out=oh[XB : XB + XP, 0:2, :],
                        in_=xhl[:, :, c * CH : (c + 1) * CH],
                    )
                    # onehot rows (dead slots cmp against 1000: always 0).
                    for k in range(NKT):
                        nc.vector.tensor_scalar(
                            out=oh[0 : RPT[k], k, :],
                            in0=doyb[0 : RPT[k], :],
                            scalar1=cmp_sb[0 : RPT[k], k : k + 1],
                            scalar2=None,
                            op0=mybir.AluOpType.is_equal,
                        )

                for h in range(2):
                    ot = ot_pool.tile([128, GRP // 2, 2 * E], F32, tag="ot")
                    ps = ps_pool.tile([128, GRP // 2, 2 * E], F32, tag="ps")
                    # Drain PSUM per 2-group quarter; 1 vector + 3 scalar
                    # copies per chunk keeps both engines under the DMA
                    # pace. Chunk 0 also DMAs per quarter (fast lead-in).
                    for q in range(2):
                        for jj in (q * 2, q * 2 + 1):
                            j = h * 4 + jj
                            nc.tensor.matmul(
                                out=ps[:, jj, :],
                                lhsT=oh[:, 0:2, j * 128 : (j + 1) * 128],
                                rhs=peW_sb[:, 0:2, :],
                                start=True,
                                stop=False,
                                perf_mode=mybir.MatmulPerfMode.DoubleRow,
                            )
                            nc.tensor.matmul(
                                out=ps[:, jj, :],
                                lhsT=oh[:, 2:4, j * 128 : (j + 1) * 128],
                                rhs=peW_sb[:, 2:4, :],
                                start=False,
                                stop=True,
                                perf_mode=mybir.MatmulPerfMode.DoubleRow,
                            )
                        lo, hi = q * 2, q * 2 + 2
                        if h == 0 and q == 0:
                            nc.vector.tensor_copy(
                                out=ot[:, lo:hi, :], in_=ps[:, lo:hi, :]
                            )
                        else:
                            nc.scalar.activation(
                                out=ot[:, lo:hi, :],
                                in_=ps[:, lo:hi, :],
                                func=mybir.ActivationFunctionType.Copy,
                            )
                        if c == 0:
                            eng = nc.sync if (h + q) % 2 == 0 else nc.scalar
                            eng.dma_start(
                                out=out5[c, h][:, lo:hi, :],
                                in_=ot[:, lo:hi, :],
                            )
                    if c != 0:
                        eng = nc.sync if h == 0 else nc.scalar
                        eng.dma_start(out=out5[c, h], in_=ot[:])
    nc.compile()
    return nc


def kernel(input_sequence, doy_sequence, W, b) -> np.ndarray:
    global _COMPILED_NC, _LAST_RESULTS

    x = np.asarray(input_sequence, dtype=np.float32)
    doy = np.asarray(doy_sequence, dtype=np.int32)
    W = np.asarray(W, dtype=np.float32)
    bias = np.asarray(b, dtype=np.float32)

    if _COMPILED_NC is None:
        _COMPILED_NC = _build()
    nc = _COMPILED_NC

    f8np = mybir.dt.np(F8)

    # Augmented weights [11, E] fp8 hi/lo: rows 0..9 = W.T, row 10 = bias.
    wTf = np.concatenate([W.T, bias[None, :]], axis=0).astype(np.float32)
    wh = wTf.astype(f8np)
    wl = (wTf - wh.astype(np.float32)).astype(f8np)

    # peW [128, 4, 512]: K-tiles 0,1 carry [wh; wl] / [wl; wh] at
    # partitions 96..117 in columns 0:256; every K-tile carries its table
    # rows at partitions 0..RPT[s] in columns 256:512 (fp8).
    petab = _make_pe()
    peWf = np.zeros((128, NKT, 2 * E), dtype=f8np)
    peWf[XB : XB + KF, 0, 0:E] = wh
    peWf[XB + KF : XB + 2 * KF, 0, 0:E] = wl
    peWf[XB : XB + KF, 1, 0:E] = wl
    peWf[XB + KF : XB + 2 * KF, 1, 0:E] = wh
    for s in range(NKT):
        peWf[0 : RPT[s], s, E : 2 * E] = petab[ROFF[s] : ROFF[s] + RPT[s]].astype(
            f8np
        )
    peWf = np.ascontiguousarray(peWf)

    # Per-partition compare constants: row id or 1000 (never matches).
    cmpc = np.full((128, NKT), 1000.0, dtype=np.float32)
    for s in range(NKT):
        for p in range(RPT[s]):
            v = ROFF[s] + p
            if v <= MAX_LEN:
                cmpc[p, s] = v
    cmpc = np.ascontiguousarray(cmpc)

    bpc = B // N_CORES
    in_maps = []
    for c in range(N_CORES):
        xc = x[c * bpc : (c + 1) * bpc].reshape(TOK, NF)
        xfull = np.zeros((KF, TOK), dtype=np.float32)
        xfull[:NF] = xc.T
        xfull[NF] = 1.0
        xh = xfull.astype(f8np)
        xl = (xfull - xh.astype(np.float32)).astype(f8np)
        combo = np.zeros((XP, TOK), dtype=f8np)  # [xh; xl; zero pad]
        combo[0:KF] = xh
        combo[KF : 2 * KF] = xl
        xhl_c = np.ascontiguousarray(
            np.broadcast_to(combo[:, None, :], (XP, 2, TOK))
        )
        doy_c = doy[c * bpc : (c + 1) * bpc].reshape(1, TOK).astype(np.float16)
        doyb0_c = np.ascontiguousarray(
            np.broadcast_to(doy_c[:, CH : (NHOST + 1) * CH], (128, NHOST * CH))
        )
        # Chunk 0's oh tile, host-built: onehot rows + [xh; xl] in tiles 0,1.
        doy0 = doy[c * bpc : (c + 1) * bpc].reshape(TOK)[:CH]
        ohp0_c = np.zeros((128, NKT, CH), dtype=f8np)
        for s in range(NKT):
            rows = ROFF[s] + np.arange(RPT[s])
            ohp0_c[0 : RPT[s], s, :] = (
                (doy0[None, :] == rows[:, None]) & (rows[:, None] <= MAX_LEN)
            ).astype(f8np)
        ohp0_c[XB : XB + KF, 0:2, :] = xh[:, None, :CH]
        ohp0_c[XB + KF : XB + 2 * KF, 0:2, :] = xl[:, None, :CH]
        ohp0_c = np.ascontiguousarray(ohp0_c)
        in_maps.append(
            {
                "xhl": xhl_c,
                "ohp0": ohp0_c,
                "peW": peWf,
                "doy": np.ascontiguousarray(doy_c),
                "doyb0": doyb0_c,
                "cmp": cmpc,
            }
        )

    _LAST_RESULTS = run_bass_kernel_spmd(nc, in_maps, core_ids=list(range(N_CORES)))

    out = np.empty((B, S, 2 * E), dtype=np.float32)
    for c in range(N_CORES):
        out[c * bpc : (c + 1) * bpc] = _LAST_RESULTS.results[c]["out"].reshape(
            bpc, S, 2 * E
        )
    return out


# revision 18
# speedup vs baseline: 1.1794x; 1.1794x over previous
"""BERT-embedding kernel for Trainium2 (8 NeuronCores, data-parallel).

Computes, for input_sequence [256,512,10], doy_sequence [256,512] (int32),
W [256,10], b [256]:

    obs = input_sequence @ W.T + b          # [256,512,256]
    pos = PE_TABLE[doy_sequence]            # [256,512,256]
    out = concat([obs, pos], axis=-1)       # [256,512,512] fp32

Strategy: shard the batch dim 8 ways (32 batches / 16384 tokens per core),
replicate W/b and the 367x256 sinusoidal PE table. The kernel is HBM-write
bound (33.5 MB of output per core) and the tensor engine is per-instruction
bound (~320 ns/matmul regardless of streamed rows, DVFS-throttled), so the
design folds the ENTIRE per-128-token computation into exactly TWO fp8
DoubleRow matmuls (2 K-tiles each = 4 K-tiles of 128 partitions):

  - K-tiles 0,1 both carry [xh; xl] fp8 at partitions 106..127 (one
    window DMA per chunk, no pad); their rhs rows are
    [wh; wl] and [wl; wh] in columns 0:256, so the two tiles sum to all
    four cross terms = (xh+xl)@(wh+wl) = obs to ~1.6e-2 absolute.
  - The 367-row table re-packs as 106/106/128/128 rows on K-tiles 0..3
    starting at partition 0 (>32-partition engine APs must start at
    partition 0; tiles 2,3 are fully compare-written so no zero pad is
    needed anywhere). The rhs rows carry the fp8 PE table in columns
    256:512 (quantization error <= 0.031 absolute = 3.9e-3 of output
    absmax; gate is 2e-2).
  - The one-hot is built by gpsimd partition_broadcast of doy (fp16)
    against per-partition row ids (DVE is_equal -> fp8; dead slots
    compare against 1000). Chunks 0..3 ship pre-broadcast from the host
    so nothing waits on the Q7 library IRAM load. The x hi/lo rows land
    in partitions 0..10 of the same SBUF tile via one small DRAM DMA per
    1024-token chunk (no engine time).
  - PSUM holds 2 half-tiles [128, 4*512] (4 banks each); the scalar
    engine drains each with ONE [128, 4, 512] copy (the vector engine
    does only compares, so next-chunk one-hots never queue behind
    copies), then a token-major 1 MB HWDGE DMA per 512 tokens
    (sync/scalar queues) writes rows as contiguous 2 KB descriptors.

Per-core DMA is ~35.5 MB ~= the output-write roofline; PE does 16
matmuls per 1024 tokens = 256 total.
"""

import math

import numpy as np

import concourse.bacc as bacc
import concourse.mybir as mybir
import concourse.tile as tile
from concourse.bass_utils import run_bass_kernel_spmd
from concourse.library_config import mlp

F32 = mybir.dt.float32
F16 = mybir.dt.float16
F8 = mybir.dt.float8e4

# Problem shapes (hardcoded per the harness contract).
B, S, NF = 256, 512, 10
E = 256
MAX_LEN = 366
N_CORES = 8
TOK = (B // N_CORES) * S          # tokens per core = 16384
CH = 1024                          # tokens per chunk
GRP = CH // 128                    # 128-token groups per chunk = 8
NCH = TOK // CH                    # 16
KF = NF + 1                        # obs rows (features + bias ones-row)
XP = 2 * KF                        # x rows ([xh; xl], no pad)
XB = 106                           # base partition of the x rows
NKT = 4                            # K-tiles (2 DoubleRow matmuls)
RPT = (106, 106, 128, 128)         # table rows per K-tile (base partition 0)
ROFF = (0, 106, 212, 340)          # first table row of each K-tile
NROW = 468                         # padded table rows (367 used)
NHOST = 2                          # chunks whose doy-broadcast ships from host

_COMPILED_NC = None
_LAST_RESULTS = None               # BassKernelResults of the most recent run


def _make_pe() -> np.ndarray:
    """Sinusoidal table, row 0 zeros (padding), rows 1..366 = positions 0..365."""
    pe = np.zeros((NROW, E), dtype=np.float32)
    position = np.arange(0, MAX_LEN, dtype=np.float32)[:, None]
    div_term = np.exp(
        np.arange(0, E, 2, dtype=np.float32) * -(math.log(10000.0) / E)
    )
    pe[1 : MAX_LEN + 1, 0::2] = np.sin(position * div_term)
    pe[1 : MAX_LEN + 1, 1::2] = np.cos(position * div_term)
    return pe


def _build():
    nc = bacc.Bacc("TRN2", target_bir_lowering=False, debug=False)
    xhl = nc.dram_tensor("xhl", [XP, 2, TOK], F8, kind="ExternalInput")
    ohp0 = nc.dram_tensor("ohp0", [128, NKT, CH], F8, kind="ExternalInput")
    peW = nc.dram_tensor("peW", [128, NKT, 2 * E], F8, kind="ExternalInput")
    doy = nc.dram_tensor("doy", [1, TOK], F16, kind="ExternalInput")
    doyb0 = nc.dram_tensor("doyb0", [128, NHOST * CH], F16, kind="ExternalInput")
    cmp = nc.dram_tensor("cmp", [128, NKT], F32, kind="ExternalInput")
    out = nc.dram_tensor("out", [TOK, 2 * E], F32, kind="ExternalOutput")

    # out viewed as [chunk, half, partition, group, 512]:
    # token ((c*2+h)*4+j)*128+p
    out5 = out.ap().rearrange("(c h j p) e -> c h p j e", p=128, j=GRP // 2, h=2)

    with tile.TileContext(nc) as tc:
        with (
            tc.tile_pool(name="const", bufs=1) as const_pool,
            tc.tile_pool(name="doyb", bufs=5) as doyb_pool,
            tc.tile_pool(name="oh", bufs=4) as oh_pool,
            tc.tile_pool(name="ot", bufs=8) as ot_pool,
            tc.tile_pool(name="ps", bufs=2, space="PSUM") as ps_pool,
        ):
            # Q7 ucode for partition_broadcast; IRAM DMA overlaps the loads.
            nc.gpsimd.load_library(mlp)

            cmp_sb = const_pool.tile([128, NKT], F32, tag="cmp_sb")
            nc.scalar.dma_start(out=cmp_sb[:], in_=cmp[:, :])
            peW_sb = const_pool.tile([128, NKT, 2 * E], F8, tag="peW_sb")
            nc.scalar.dma_start(out=peW_sb[:], in_=peW[:, :, :])
            doyb0_sb = const_pool.tile([128, NHOST * CH], F16, tag="doyb0_sb")
            nc.scalar.dma_start(out=doyb0_sb[:], in_=doyb0[:, :])
            doy_sb = const_pool.tile([1, TOK], F16, tag="doy_sb")
            nc.sync.dma_start(out=doy_sb[:], in_=doy[:, :])

            for c in range(NCH):
                oh = oh_pool.tile([128, NKT, CH], F8, tag="oh")
                if c == 0:
                    # Chunk 0's whole K-tile (onehot + x rows) ships
                    # host-built: one DMA, no compare latency at startup.
                    nc.sync.dma_start(out=oh[:], in_=ohp0[:, :, :])
                else:
                    # doyb[p, t] = doy[t] for the chunk's tokens, fp16.
                    if c - 1 < NHOST:
                        doyb = doyb0_sb[:, (c - 1) * CH : c * CH]
                    else:
                        dt = doyb_pool.tile([128, CH], F16, tag="doyb")
                        nc.gpsimd.partition_broadcast(
                            dt[:], doy_sb[0:1, c * CH : (c + 1) * CH]
                        )
                        doyb = dt[:]
                    # [xh; xl] fp8 rows into partitions 106..127 of
                    # K-tiles 0,1 (pure DMA).
                    nc.sync.dma_start(
                        out=oh[XB : XB + XP, 0:2, :],
                        in_=xhl[:, :, c * CH : (c + 1) * CH],
                    )
                    # onehot rows (dead slots cmp against 1000: always 0).
                    for k in range(NKT):
                        nc.vector.tensor_scalar(
                            out=oh[0 : RPT[k], k, :],
                            in0=doyb[0 : RPT[k], :],
                            scalar1=cmp_sb[0 : RPT[k], k : k + 1],
                            scalar2=None,
                            op0=mybir.AluOpType.is_equal,
                        )

                for h in range(2):
                    ot = ot_pool.tile([128, GRP // 2, 2 * E], F32, tag="ot")
                    ps = ps_pool.tile([128, GRP // 2, 2 * E], F32, tag="ps")
                    # the first chunk drains per 2-group quarter so its
                    # output DMA fires as early as possible.
                    nq = 2 if c == 0 else 1
                    for q in range(nq):
                        for jj in range(q * 4 // nq, (q + 1) * 4 // nq):
                            j = h * 4 + jj
                            nc.tensor.matmul(
                                out=ps[:, jj, :],
                                lhsT=oh[:, 0:2, j * 128 : (j + 1) * 128],
                                rhs=peW_sb[:, 0:2, :],
                                start=True,
                                stop=False,
                                perf_mode=mybir.MatmulPerfMode.DoubleRow,
                            )
                            nc.tensor.matmul(
                                out=ps[:, jj, :],
                                lhsT=oh[:, 2:4, j * 128 : (j + 1) * 128],
                                rhs=peW_sb[:, 2:4, :],
                                start=False,
                                stop=True,
                                perf_mode=mybir.MatmulPerfMode.DoubleRow,
                            )
                        lo, hi = q * 4 // nq, (q + 1) * 4 // nq
                        nc.scalar.activation(
                            out=ot[:, lo:hi, :],
                            in_=ps[:, lo:hi, :],
                            func=mybir.ActivationFunctionType.Copy,
                        )
                        eng = nc.sync if (h + q) % 2 == 0 else nc.scalar
                        eng.dma_start(
                            out=out5[c, h][:, lo:hi, :], in_=ot[:, lo:hi, :]
                        )
    nc.compile()
    return nc


def kernel(input_sequence, doy_sequence, W, b) -> np.ndarray:
    global _COMPILED_NC, _LAST_RESULTS

    x = np.asarray(input_sequence, dtype=np.float32)
    doy = np.asarray(doy_sequence, dtype=np.int32)
    W = np.asarray(W, dtype=np.float32)
    bias = np.asarray(b, dtype=np.float32)

    if _COMPILED_NC is None:
        _COMPILED_NC = _build()
    nc = _COMPILED_NC

    f8np = mybir.dt.np(F8)

    # Augmented weights [11, E] fp8 hi/lo: rows 0..9 = W.T, row 10 = bias.
    wTf = np.concatenate([W.T, bias[None, :]], axis=0).astype(np.float32)
    wh = wTf.astype(f8np)
    wl = (wTf - wh.astype(np.float32)).astype(f8np)

    # peW [128, 4, 512]: K-tiles 0,1 carry [wh; wl] / [wl; wh] at
    # partitions 96..117 in columns 0:256; every K-tile carries its table
    # rows at partitions 0..RPT[s] in columns 256:512 (fp8).
    petab = _make_pe()
    peWf = np.zeros((128, NKT, 2 * E), dtype=f8np)
    peWf[XB : XB + KF, 0, 0:E] = wh
    peWf[XB + KF : XB + 2 * KF, 0, 0:E] = wl
    peWf[XB : XB + KF, 1, 0:E] = wl
    peWf[XB + KF : XB + 2 * KF, 1, 0:E] = wh
    for s in range(NKT):
        peWf[0 : RPT[s], s, E : 2 * E] = petab[ROFF[s] : ROFF[s] + RPT[s]].astype(
            f8np
        )
    peWf = np.ascontiguousarray(peWf)

    # Per-partition compare constants: row id or 1000 (never matches).
    cmpc = np.full((128, NKT), 1000.0, dtype=np.float32)
    for s in range(NKT):
        for p in range(RPT[s]):
            v = ROFF[s] + p
            if v <= MAX_LEN:
                cmpc[p, s] = v
    cmpc = np.ascontiguousarray(cmpc)

    bpc = B // N_CORES
    in_maps = []
    for c in range(N_CORES):
        xc = x[c * bpc : (c + 1) * bpc].reshape(TOK, NF)
        xfull = np.zeros((KF, TOK), dtype=np.float32)
        xfull[:NF] = xc.T
        xfull[NF] = 1.0
        xh = xfull.astype(f8np)
        xl = (xfull - xh.astype(np.float32)).astype(f8np)
        combo = np.zeros((XP, TOK), dtype=f8np)  # [xh; xl; zero pad]
        combo[0:KF] = xh
        combo[KF : 2 * KF] = xl
        xhl_c = np.ascontiguousarray(
            np.broadcast_to(combo[:, None, :], (XP, 2, TOK))
        )
        doy_c = doy[c * bpc : (c + 1) * bpc].reshape(1, TOK).astype(np.float16)
        doyb0_c = np.ascontiguousarray(
            np.broadcast_to(doy_c[:, CH : (NHOST + 1) * CH], (128, NHOST * CH))
        )
        # Chunk 0's oh tile, host-built: onehot rows + [xh; xl] in tiles 0,1.
        doy0 = doy[c * bpc : (c + 1) * bpc].reshape(TOK)[:CH]
        ohp0_c = np.zeros((128, NKT, CH), dtype=f8np)
        for s in range(NKT):
            rows = ROFF[s] + np.arange(RPT[s])
            ohp0_c[0 : RPT[s], s, :] = (
                (doy0[None, :] == rows[:, None]) & (rows[:, None] <= MAX_LEN)
            ).astype(f8np)
        ohp0_c[XB : XB + KF, 0:2, :] = xh[:, None, :CH]
        ohp0_c[XB + KF : XB + 2 * KF, 0:2, :] = xl[:, None, :CH]
        ohp0_c = np.ascontiguousarray(ohp0_c)
        in_maps.append(
            {
                "xhl": xhl_c,
                "ohp0": ohp0_c,
                "peW": peWf,
                "doy": np.ascontiguousarray(doy_c),
                "doyb0": doyb0_c,
                "cmp": cmpc,
            }
        )

    _LAST_RESULTS = run_bass_kernel_spmd(nc, in_maps, core_ids=list(range(N_CORES)))

    out = np.empty((B, S, 2 * E), dtype=np.float32)
    for c in range(N_CORES):
        out[c * bpc : (c + 1) * bpc] = _LAST_RESULTS.results[c]["out"].reshape(
            bpc, S, 2 * E
        )
    return out


# revision 19
# speedup vs baseline: 1.1826x; 1.0027x over previous
"""BERT-embedding kernel for Trainium2 (8 NeuronCores, data-parallel).

Computes, for input_sequence [256,512,10], doy_sequence [256,512] (int32),
W [256,10], b [256]:

    obs = input_sequence @ W.T + b          # [256,512,256]
    pos = PE_TABLE[doy_sequence]            # [256,512,256]
    out = concat([obs, pos], axis=-1)       # [256,512,512] fp32

Strategy: shard the batch dim 8 ways (32 batches / 16384 tokens per core),
replicate W/b and the 367x256 sinusoidal PE table. The kernel is HBM-write
bound (33.5 MB of output per core) and the tensor engine is per-instruction
bound (~320 ns/matmul regardless of streamed rows, DVFS-throttled), so the
design folds the ENTIRE per-128-token computation into exactly TWO fp8
DoubleRow matmuls (2 K-tiles each = 4 K-tiles of 128 partitions):

  - K-tiles 0,1 both carry [xh; xl] fp8 at partitions 106..127 (one
    window DMA per chunk, no pad); their rhs rows are
    [wh; wl] and [wl; wh] in columns 0:256, so the two tiles sum to all
    four cross terms = (xh+xl)@(wh+wl) = obs to ~1.6e-2 absolute.
  - The 367-row table re-packs as 106/106/128/128 rows on K-tiles 0..3
    starting at partition 0 (>32-partition engine APs must start at
    partition 0; tiles 2,3 are fully compare-written so no zero pad is
    needed anywhere). The rhs rows carry the fp8 PE table in columns
    256:512 (quantization error <= 0.031 absolute = 3.9e-3 of output
    absmax; gate is 2e-2).
  - The one-hot is built by gpsimd partition_broadcast of doy (fp16)
    against per-partition row ids (DVE is_equal -> fp8; dead slots
    compare against 1000). Chunks 0..3 ship pre-broadcast from the host
    so nothing waits on the Q7 library IRAM load. The x hi/lo rows land
    in partitions 0..10 of the same SBUF tile via one small DRAM DMA per
    1024-token chunk (no engine time).
  - PSUM holds 2 half-tiles [128, 4*512] (4 banks each); the scalar
    engine drains each with ONE [128, 4, 512] copy (the vector engine
    does only compares, so next-chunk one-hots never queue behind
    copies), then a token-major 1 MB HWDGE DMA per 512 tokens
    (sync/scalar queues) writes rows as contiguous 2 KB descriptors.

Per-core DMA is ~35.5 MB ~= the output-write roofline; PE does 16
matmuls per 1024 tokens = 256 total.
"""

import math

import numpy as np

import concourse.bacc as bacc
import concourse.mybir as mybir
import concourse.tile as tile
from concourse.bass_utils import run_bass_kernel_spmd
from concourse.library_config import mlp

F32 = mybir.dt.float32
F16 = mybir.dt.float16
F8 = mybir.dt.float8e4

# Problem shapes (hardcoded per the harness contract).
B, S, NF = 256, 512, 10
E = 256
MAX_LEN = 366
N_CORES = 8
TOK = (B // N_CORES) * S          # tokens per core = 16384
CH = 1024                          # tokens per chunk
GRP = CH // 128                    # 128-token groups per chunk = 8
NCH = TOK // CH                    # 16
KF = NF + 1                        # obs rows (features + bias ones-row)
XP = 2 * KF                        # x rows ([xh; xl], no pad)
XB = 106                           # base partition of the x rows
NKT = 4                            # K-tiles (2 DoubleRow matmuls)
RPT = (106, 106, 128, 128)         # table rows per K-tile (base partition 0)
ROFF = (0, 106, 212, 340)          # first table row of each K-tile
NROW = 468                         # padded table rows (367 used)
NHOST = 2                          # chunks whose doy-broadcast ships from host

_COMPILED_NC = None
_LAST_RESULTS = None               # BassKernelResults of the most recent run


def _make_pe() -> np.ndarray:
    """Sinusoidal table, row 0 zeros (padding), rows 1..366 = positions 0..365."""
    pe = np.zeros((NROW, E), dtype=np.float32)
    position = np.arange(0, MAX_LEN, dtype=np.float32)[:, None]
    div_term = np.exp(
        np.arange(0, E, 2, dtype=np.float32) * -(math.log(10000.0) / E)
    )
    pe[1 : MAX_LEN + 1, 0::2] = np.sin(position * div_term)
    pe[1 : MAX_LEN + 1, 1::2] = np.cos(position * div_term)
    return pe


def _build():
    nc = bacc.Bacc("TRN2", target_bir_lowering=False, debug=False)
    xhl = nc.dram_tensor("xhl", [XP, 2, TOK], F8, kind="ExternalInput")
    ohp0 = nc.dram_tensor("ohp0", [128, NKT, CH], F8, kind="ExternalInput")
    peW = nc.dram_tensor("peW", [128, NKT, 2 * E], F8, kind="ExternalInput")
    doy = nc.dram_tensor("doy", [1, TOK], F16, kind="ExternalInput")
    doyb0 = nc.dram_tensor("doyb0", [128, NHOST * CH], F16, kind="ExternalInput")
    cmp = nc.dram_tensor("cmp", [128, NKT], F32, kind="ExternalInput")
    out = nc.dram_tensor("out", [TOK, 2 * E], F32, kind="ExternalOutput")

    # out viewed as [chunk, half, partition, group, 512]:
    # token ((c*2+h)*4+j)*128+p
    out5 = out.ap().rearrange("(c h j p) e -> c h p j e", p=128, j=GRP // 2, h=2)

    with tile.TileContext(nc) as tc:
        with (
            tc.tile_pool(name="const", bufs=1) as const_pool,
            tc.tile_pool(name="doyb", bufs=5) as doyb_pool,
            tc.tile_pool(name="oh", bufs=3) as oh_pool,
            tc.tile_pool(name="ot", bufs=6) as ot_pool,
            tc.tile_pool(name="ps", bufs=2, space="PSUM") as ps_pool,
        ):
            # Q7 ucode for partition_broadcast; IRAM DMA overlaps the loads.
            nc.gpsimd.load_library(mlp)

            cmp_sb = const_pool.tile([128, NKT], F32, tag="cmp_sb")
            nc.scalar.dma_start(out=cmp_sb[:], in_=cmp[:, :])
            peW_sb = const_pool.tile([128, NKT, 2 * E], F8, tag="peW_sb")
            nc.scalar.dma_start(out=peW_sb[:], in_=peW[:, :, :])
            doyb0_sb = const_pool.tile([128, NHOST * CH], F16, tag="doyb0_sb")
            nc.scalar.dma_start(out=doyb0_sb[:], in_=doyb0[:, :])
            doy_sb = const_pool.tile([1, TOK], F16, tag="doy_sb")
            nc.sync.dma_start(out=doy_sb[:], in_=doy[:, :])

            for c in range(NCH):
                oh = oh_pool.tile([128, NKT, CH], F8, tag="oh")
                if c == 0:
                    # Chunk 0's whole K-tile (onehot + x rows) ships
                    # host-built: one DMA, no compare latency at startup.
                    nc.sync.dma_start(out=oh[:], in_=ohp0[:, :, :])
                else:
                    # doyb[p, t] = doy[t] for the chunk's tokens, fp16.
                    if c - 1 < NHOST:
                        doyb = doyb0_sb[:, (c - 1) * CH : c * CH]
                    else:
                        dt = doyb_pool.tile([128, CH], F16, tag="doyb")
                        nc.gpsimd.partition_broadcast(
                            dt[:], doy_sb[0:1, c * CH : (c + 1) * CH]
                        )
                        doyb = dt[:]
                    # [xh; xl] fp8 rows into partitions 106..127 of
                    # K-tiles 0,1 (pure DMA).
                    nc.sync.dma_start(
                        out=oh[XB : XB + XP, 0:2, :],
                        in_=xhl[:, :, c * CH : (c + 1) * CH],
                    )
                    # onehot rows (dead slots cmp against 1000: always 0).
                    for k in range(NKT):
                        nc.vector.tensor_scalar(
                            out=oh[0 : RPT[k], k, :],
                            in0=doyb[0 : RPT[k], :],
                            scalar1=cmp_sb[0 : RPT[k], k : k + 1],
                            scalar2=None,
                            op0=mybir.AluOpType.is_equal,
                        )

                for h in range(2):
                    ot = ot_pool.tile([128, GRP // 2, 2 * E], F32, tag="ot")
                    ps = ps_pool.tile([128, GRP // 2, 2 * E], F32, tag="ps")
                    # the first chunk drains per 2-group quarter so its
                    # output DMA fires as early as possible.
                    nq = 2 if c == 0 else 1
                    for q in range(nq):
                        for jj in range(q * 4 // nq, (q + 1) * 4 // nq):
                            j = h * 4 + jj
                            nc.tensor.matmul(
                                out=ps[:, jj, :],
                                lhsT=oh[:, 0:2, j * 128 : (j + 1) * 128],
                                rhs=peW_sb[:, 0:2, :],
                                start=True,
                                stop=False,
                                perf_mode=mybir.MatmulPerfMode.DoubleRow,
                            )
                            nc.tensor.matmul(
                                out=ps[:, jj, :],
                                lhsT=oh[:, 2:4, j * 128 : (j + 1) * 128],
                                rhs=peW_sb[:, 2:4, :],
                                start=False,
                                stop=True,
                                perf_mode=mybir.MatmulPerfMode.DoubleRow,
                            )
                        lo, hi = q * 4 // nq, (q + 1) * 4 // nq
                        nc.scalar.activation(
                            out=ot[:, lo:hi, :],
                            in_=ps[:, lo:hi, :],
                            func=mybir.ActivationFunctionType.Copy,
                        )
                        eng = nc.sync if (h + q) % 2 == 0 else nc.scalar
                        eng.dma_start(
                            out=out5[c, h][:, lo:hi, :], in_=ot[:, lo:hi, :]
                        )
    nc.compile()
    return nc


def kernel(input_sequence, doy_sequence, W, b) -> np.ndarray:
    global _COMPILED_NC, _LAST_RESULTS

    x = np.asarray(input_sequence, dtype=np.float32)
    doy = np.asarray(doy_sequence, dtype=np.int32)
    W = np.asarray(W, dtype=np.float32)
    bias = np.asarray(b, dtype=np.float32)

    if _COMPILED_NC is None:
        _COMPILED_NC = _build()
    nc = _COMPILED_NC

    f8np = mybir.dt.np(F8)

    # Augmented weights [11, E] fp8 hi/lo: rows 0..9 = W.T, row 10 = bias.
    wTf = np.concatenate([W.T, bias[None, :]], axis=0).astype(np.float32)
    wh = wTf.astype(f8np)
    wl = (wTf - wh.astype(np.float32)).astype(f8np)

    # peW [128, 4, 512]: K-tiles 0,1 carry [wh; wl] / [wl; wh] at
    # partitions 96..117 in columns 0:256; every K-tile carries its table
    # rows at partitions 0..RPT[s] in columns 256:512 (fp8).
    petab = _make_pe()
    peWf = np.zeros((128, NKT, 2 * E), dtype=f8np)
    peWf[XB : XB + KF, 0, 0:E] = wh
    peWf[XB + KF : XB + 2 * KF, 0, 0:E] = wl
    peWf[XB : XB + KF, 1, 0:E] = wl
    peWf[XB + KF : XB + 2 * KF, 1, 0:E] = wh
    for s in range(NKT):
        peWf[0 : RPT[s], s, E : 2 * E] = petab[ROFF[s] : ROFF[s] + RPT[s]].astype(
            f8np
        )
    peWf = np.ascontiguousarray(peWf)

    # Per-partition compare constants: row id or 1000 (never matches).
    cmpc = np.full((128, NKT), 1000.0, dtype=np.float32)
    for s in range(NKT):
        for p in range(RPT[s]):
            v = ROFF[s] + p
            if v <= MAX_LEN:
                cmpc[p, s] = v
    cmpc = np.ascontiguousarray(cmpc)

    bpc = B // N_CORES
    in_maps = []
    for c in range(N_CORES):
        xc = x[c * bpc : (c + 1) * bpc].reshape(TOK, NF)
        xfull = np.zeros((KF, TOK), dtype=np.float32)
        xfull[:NF] = xc.T
        xfull[NF] = 1.0
        xh = xfull.astype(f8np)
        xl = (xfull - xh.astype(np.float32)).astype(f8np)
        combo = np.zeros((XP, TOK), dtype=f8np)  # [xh; xl; zero pad]
        combo[0:KF] = xh
        combo[KF : 2 * KF] = xl
        xhl_c = np.ascontiguousarray(
            np.broadcast_to(combo[:, None, :], (XP, 2, TOK))
        )
        doy_c = doy[c * bpc : (c + 1) * bpc].reshape(1, TOK).astype(np.float16)
        doyb0_c = np.ascontiguousarray(
            np.broadcast_to(doy_c[:, CH : (NHOST + 1) * CH], (128, NHOST * CH))
        )
        # Chunk 0's oh tile, host-built: onehot rows + [xh; xl] in tiles 0,1.
        doy0 = doy[c * bpc : (c + 1) * bpc].reshape(TOK)[:CH]
        ohp0_c = np.zeros((128, NKT, CH), dtype=f8np)
        for s in range(NKT):
            rows = ROFF[s] + np.arange(RPT[s])
            ohp0_c[0 : RPT[s], s, :] = (
                (doy0[None, :] == rows[:, None]) & (rows[:, None] <= MAX_LEN)
            ).astype(f8np)
        ohp0_c[XB : XB + KF, 0:2, :] = xh[:, None, :CH]
        ohp0_c[XB + KF : XB + 2 * KF, 0:2, :] = xl[:, None, :CH]
        ohp0_c = np.ascontiguousarray(ohp0_c)
        in_maps.append(
            {
                "xhl": xhl_c,
                "ohp0": ohp0_c,
                "peW": peWf,
                "doy": np.ascontiguousarray(doy_c),
                "doyb0": doyb0_c,
                "cmp": cmpc,
            }
        )

    _LAST_RESULTS = run_bass_kernel_spmd(nc, in_maps, core_ids=list(range(N_CORES)))

    out = np.empty((B, S, 2 * E), dtype=np.float32)
    for c in range(N_CORES):
        out[c * bpc : (c + 1) * bpc] = _LAST_RESULTS.results[c]["out"].reshape(
            bpc, S, 2 * E
        )
    return out
